# revision 85
# baseline (speedup 1.0000x reference)
"""Trainium2 Bass kernel for nn_KerasCustomMappingLayer (osu-style map construction).

Strategy (pure data-parallel over 8 NeuronCores, B=1048576 rows):
  - fp16 end-to-end on device, plus int8 (global scale) for the raw inputs
    of rerand steps whose carry is never read (HW-validated: rel err 5.4e-3
    vs f32 reference, gate is 2e-2). Cuts HBM traffic ~2.2x and unlocks
    DVE 2x/4x perf modes.
  - Host stages inputs into a column-major SBUF image [T][P][C_IN][F] with a
    computation-aware column permutation, so every on-device operand slice is
    contiguous along the row axis (packed, stride-1) -> DVE fast modes apply.
  - All 10 per-step scalars are host-known at build time; each scan step is
    specialized on (rerand, is_slider). Positions kept in the scaled domain
    x/XMAX, y/YMAX.
  - rsqrt via custom DVE hypot2 (f32 accum) + ACT Exp(-0.5*Ln(x)).
  - Wall clamp is 2 custom DVE ops per axis with the relu folded in:
      t  = px + ((px<wl) - 0.5)*relu(c * -2lk)
      x' = t  + (0.5 - (t>wr))*relu(c * +2lk)
  - Normalized cos/sin are materialized pre-scaled by slider length where
    consumed by slider steps (scale absorbed into downstream lincomb consts).
  - Circle-step c4c5 (exact duplicates of c0c1) are fanned out host-side
    during unsharding; device computes every distinct output value once.
"""
import sys
import numpy as np

for _p in ("/opt/trn_rl_repo",):
    if _p not in sys.path:
        sys.path.insert(0, _p)

NGS = 10
XMAX, YMAX = 512.0, 384.0
LMUL, MTFD = 1.0, 1.0
N_CORES = 8
P = 128

_OPS = {}
_NC_CACHE = {}
_FS_CACHE = {}


def _get_custom_ops():
    global _OPS
    if _OPS:
        return _OPS
    import concourse.dve_ops as dve_ops
    from concourse.dve_spec import Spec, Src0, Src1, C0, C1, C2, relu, sq
    from concourse.dve_uop import DveOpSpec

    defs = {
        "ANT_HYPOT2": dict(
            body=sq(Src0) + sq(Src1),
            reference=lambda in0, in1, s0, s1, imm2: (
                in0.astype(np.float32) ** 2 + in1.astype(np.float32) ** 2),
        ),
        # t = px + ((px<wl) - 0.5)*relu(c*imm2)        (imm2 = -2*lk)
        "ANT_WALLF1": dict(
            body=Src0 + ((Src0 < C0) - C1) * relu(Src1 * C2),
            reference=lambda in0, in1, s0, s1, imm2: (
                in0.astype(np.float32)
                + ((in0 < s0) - np.float32(s1))
                * np.maximum(in1.astype(np.float32) * np.float32(imm2), 0)),
        ),
        # x = t + (0.5 - (t>wr))*relu(c*imm2)          (imm2 = +2*lk)
        "ANT_WALLF2": dict(
            body=Src0 + (C1 - (Src0 > C0)) * relu(Src1 * C2),
            reference=lambda in0, in1, s0, s1, imm2: (
                in0.astype(np.float32)
                + (np.float32(s1) - (in0 > s0))
                * np.maximum(in1.astype(np.float32) * np.float32(imm2), 0)),
        ),
        "ANT_LINCOMB": dict(
            body=Src0 * C0 + Src1 * C1,
            reference=lambda in0, in1, s0, s1, imm2: (
                in0.astype(np.float32) * s0 + in1.astype(np.float32) * s1),
        ),
    }
    ops = {}
    for name, d in defs.items():
        existing = next((o for o in dve_ops.OPS if o.name == name), None)
        if existing is not None:
            ops[name] = existing
            continue
        spec = Spec(body=d["body"], reference=d["reference"])
        row = max(dve_ops._SUB_OPCODE_FOR_NAME.values()) + 1
        assert row < 0x20, "custom DVE row overflow"
        dve_ops._SUB_OPCODE_FOR_NAME[name] = row
        shas = {}
        for ver in ("v3", "v4"):
            try:
                uops = dve_ops.lower(spec, ver=ver)
                shas[ver] = DveOpSpec(
                    name=name, opcode=row, uops=uops,
                    rd1_en=dve_ops.has_src1(spec)).sha(ver)
            except Exception:
                pass
        assert shas, f"lower() failed for {name}"
        op = dve_ops.DveOp(name, spec, subdim=False, uops_sha=shas)
        dve_ops.OPS.append(op)
        dve_ops.CUSTOM_DVE_SPECS[name] = spec
        ops[name] = op
    _OPS = ops
    return ops


def _host_consts(slider_lengths, slider_cos_each, slider_sin_each,
                 note_distances, tick_diff, start_pos, is_slider):
    f = np.float32
    l = (f(LMUL) * note_distances.astype(f)).astype(f)
    rr = tuple(bool(x) for x in (tick_diff.astype(f) > f(MTFD)))
    isl = tuple(bool(x) for x in (np.asarray(is_slider) != 0))
    return dict(
        wl=tuple(float(x) for x in (f(0.05 * XMAX) + l * f(0.5)) / f(XMAX)),
        wr=tuple(float(x) for x in (f(0.95 * XMAX) - l * f(0.5)) / f(XMAX)),
        wt=tuple(float(x) for x in (f(0.05 * YMAX) + l * f(0.5)) / f(YMAX)),
        wb=tuple(float(x) for x in (f(0.95 * YMAX) - l * f(0.5)) / f(YMAX)),
        lkx=tuple(float(x) for x in l / f(XMAX)),
        lky=tuple(float(x) for x in l / f(YMAX)),
        rr=rr,
        isl=isl,
        slnx=tuple(float(x) for x in slider_lengths.astype(f) / f(XMAX)),
        slny=tuple(float(x) for x in slider_lengths.astype(f) / f(YMAX)),
        scos=tuple(float(x) for x in slider_cos_each.astype(f)),
        ssin=tuple(float(x) for x in slider_sin_each.astype(f)),
        px0=float(f(start_pos[0]) / f(XMAX)),
        py0=float(f(start_pos[1]) / f(YMAX)),
    )


def _layout(c, use_int8=True):
    """Derive step classification, input/output column permutations and
    run-groupings from the host constants."""
    rr, isl = c["rr"], c["isl"]
    W = [k for k in range(NGS) if not rr[k]]          # wall steps
    S = [k for k in range(NGS) if isl[k]]             # slider steps
    R = [k for k in range(NGS) if rr[k]]              # rerand steps
    CR1 = [k for k in range(NGS) if (not isl[k]) and rr[k]]   # circle&rerand
    CR0 = [k for k in range(NGS) if (not isl[k]) and not rr[k]]  # circle&wall
    r0, s, ncr1, r1 = len(W), len(S), len(CR1), len(R)

    # rerand steps whose (px,py) carry is never read (next step not a wall):
    # their raw pair can be shipped as int8 (global scale, dequant folded
    # into the ACT Copy's scale immediate).
    R8 = [k for k in R
          if use_int8 and ((k + 1 >= NGS) or ((k + 1) not in W))]
    R16 = [k for k in R if k not in R8]

    # pairs needing normalization, in RAWC/RAWS region order
    pairs = [(k, 0) for k in W] + [(k, 1) for k in S] + [(k, 1) for k in CR1]
    nn = len(pairs)
    # source var columns: (k,0)->cos col k, sin col 20+k; (k,1)->10+k, 30+k
    rawc_src = [k + 10 * hi for (k, hi) in pairs]
    raws_src = [20 + k + 10 * hi for (k, hi) in pairs]
    rer_src = []
    for k in R16:
        rer_src += [k, 20 + k]
    in_perm = rawc_src + raws_src + rer_src          # staged fp16 columns
    in8_perm = []
    for k in R8:
        in8_perm += [k, 20 + k]                      # staged int8 columns
    c_in = len(in_perm)
    c_in8 = len(in8_perm)
    rer_off = 2 * nn

    # device output columns, readiness-ordered blocks:
    #   A: rerand c0c1 pairs (R order)    — ready right after input lands
    #   B: circle&rerand c2c3 pairs        — ready after Exp + muls
    #   C: wall c0c1, CR0 c2c3, slider c2..c5 — ready last
    # circle-step c4c5 duplicates are fanned out host-side.
    dev_cols = []
    col_of = {}

    def _add(k, m):
        col_of[(k, m)] = len(dev_cols)
        dev_cols.append((k, m))

    for k in range(NGS):
        nm = 4 if (k in CR1 or k in CR0) else 6
        for m in range(nm):
            _add(k, m)
    c_out = len(dev_cols)
    third = (c_out + 2) // 3
    out_chunks = sorted({min(third, c_out), min(2 * third, c_out), c_out})

    # index helpers into region orders
    wall_idx = {k: i for i, k in enumerate(W)}
    slider_idx = {k: i for i, k in enumerate(S)}
    cr1_idx = {k: i for i, k in enumerate(CR1)}
    rer_idx = {k: i for i, k in enumerate(R)}

    def _uniform_runs(steps, m):
        """Group region indices [0..len) into runs with uniform out-col
        stride (source region is contiguous in region order by layout)."""
        out = []
        n = len(steps)
        i = 0
        while i < n:
            stride = None
            j = i
            while j + 1 < n:
                st = col_of[(steps[j + 1], m)] - col_of[(steps[j], m)]
                if stride is None:
                    stride = st
                if st != stride or st <= 0:
                    break
                j += 1
            if stride is None or stride <= 0:
                stride = 1
            out.append((i, j - i + 1, col_of[(steps[i], m)], stride))
            i = j + 1
        return out

    rer_runs8 = _uniform_runs(R8, 0)   # (region_start, n, out_col0, stride)
    rer_runs16 = _uniform_runs(R16, 0)
    cr1_runs = _uniform_runs(CR1, 2)

    return dict(W=W, S=S, R=R, R8=R8, R16=R16, CR1=CR1, CR0=CR0, nn=nn,
                c_in=c_in, c_in8=c_in8, c_out=c_out, rer_off=rer_off,
                in_perm=in_perm, in8_perm=in8_perm,
                dev_cols=dev_cols, col_of=col_of, wall_idx=wall_idx,
                slider_idx=slider_idx, cr1_idx=cr1_idx, rer_idx=rer_idx,
                rer_runs8=rer_runs8, rer_runs16=rer_runs16,
                cr1_runs=cr1_runs, out_chunks=out_chunks)


def _build(c, lay, b_core, fs=None, in_bufs=3, cr1_eng="pool",
           adds_eng="vector", work_bufs=2, io_bufs=2, out_split=1,
           rer_eng="act", in_split=False, p0_split=False, out_chunks=None,
           cr0_eng="pool", q_scale=None, rer16_eng=None, p0_first=False,
           out_chunks_last=None, in_dma_eng="sync"):
    import concourse.bacc as bacc
    import concourse.mybir as mybir
    from concourse.tile import TileContext

    f16 = mybir.dt.float16
    f32 = mybir.dt.float32
    AF = mybir.ActivationFunctionType
    ops = _get_custom_ops()
    HYP, LIN = ops["ANT_HYPOT2"], ops["ANT_LINCOMB"]
    Q1, Q2 = ops["ANT_WALLF1"], ops["ANT_WALLF2"]

    W, S, R = lay["W"], lay["S"], lay["R"]
    CR1, CR0 = lay["CR1"], lay["CR0"]
    nn, c_in, c_out = lay["nn"], lay["c_in"], lay["c_out"]
    c_in8 = lay["c_in8"]
    use_i8 = c_in8 > 0
    assert (not use_i8) or q_scale is not None, \
        "layout has int8 columns; _build needs q_scale"
    rer_off = lay["rer_off"]
    col_of = lay["col_of"]
    wall_idx, slider_idx = lay["wall_idx"], lay["slider_idx"]
    cr1_idx, rer_idx = lay["cr1_idx"], lay["rer_idx"]
    r0, s = len(W), len(S)
    mm = r0 + s

    npp = b_core // P
    if fs is None:
        n_tiles = 4
        base, rem = divmod(npp, n_tiles)
        fs = [base + (1 if t < rem else 0) for t in range(n_tiles)]
    Fs = list(fs)
    assert sum(Fs) == npp
    n_tiles = len(Fs)

    nc = bacc.Bacc("TRN2", target_bir_lowering=False, debug=False)
    Fmax = max(Fs)
    var = nc.dram_tensor("var", [n_tiles * P, c_in, Fmax], f16,
                         kind="ExternalInput")
    var8 = nc.dram_tensor("var8", [n_tiles * P, max(c_in8, 1), Fmax],
                          mybir.dt.int8,
                          kind="ExternalInput") if use_i8 else None
    out = nc.dram_tensor("out", [n_tiles * P, c_out, Fmax], f16,
                         kind="ExternalOutput")

    rn_bufs = n_tiles if p0_first else work_bufs
    with TileContext(nc) as tc:
        with tc.tile_pool(name="in", bufs=in_bufs) as inp, \
             tc.tile_pool(name="io", bufs=io_bufs) as iop, \
             tc.tile_pool(name="work", bufs=work_bufs) as wp, \
             tc.tile_pool(name="rnp", bufs=rn_bufs) as rnp, \
             tc.tile_pool(name="cst", bufs=1) as cp:
            px0t = cp.tile([P, Fmax], f16, tag="px0")
            py0t = cp.tile([P, Fmax], f16, tag="py0")
            nc.vector.memset(px0t[:], c["px0"])
            nc.vector.memset(py0t[:], c["py0"])
            czero = cp.tile([P, 1], f32, tag="czero")
            nc.vector.memset(czero[:], 0.0)
            nc.const_aps.aps[(f32, 0.0)] = czero[:]

            # One activation table holds Ln+Exp+Copy; preload it so the
            # compiler's fixpoint pass doesn't thrash between per-function
            # tables (1283ns per reload, on the phase-0 critical path).
            from concourse.hw_specs import get_activation_tables
            needed = {AF.Ln, AF.Exp, AF.Copy}
            tabs = list(get_activation_tables(nc.m.arch).items())
            tid = next(i for i, (_, fset) in enumerate(tabs)
                       if needed <= fset)
            nc.scalar.add_instruction(mybir.InstLoadActFuncSet(
                name=nc.get_next_instruction_name(), act_func_set_id=tid,
                ins=[], outs=[]))

            def emit_phase0(t, F):
                in_eng = (nc.gpsimd if in_dma_eng == "pool"
                          or (in_dma_eng == "first_pool" and t == 0)
                          else nc.sync)
                tin = inp.tile([P, c_in, F], f16, tag="tin", name="tin")
                if in_split and rer_off < c_in:
                    in_eng.dma_start(tin[:, 0:rer_off, :],
                                     var[t * P:(t + 1) * P, 0:rer_off, 0:F])
                    in_eng.dma_start(tin[:, rer_off:c_in, :],
                                     var[t * P:(t + 1) * P, rer_off:c_in,
                                         0:F])
                else:
                    in_eng.dma_start(tin[:],
                                     var[t * P:(t + 1) * P, :, 0:F])
                tin8 = None
                if use_i8:
                    tin8 = inp.tile([P, c_in8, F], mybir.dt.int8, tag="tin8",
                                    name="tin8")
                    in_eng.dma_start(tin8[:],
                                     var8[t * P:(t + 1) * P, :, 0:F])
                ssum = wp.tile([P, nn, F], f32, tag="ssum",
                               name="ssum") if nn else None
                rn = rnp.tile([P, nn, F], f16, tag="rn",
                              name="rn") if nn else None

                # ---- phase 0: rn = (c^2+s^2)^-0.5 (f32 accum, fp16 out) ----
                p0_splits = [x for x in (mm, nn) if x > 0]
                if p0_splits and p0_splits[0] == p0_splits[-1]:
                    p0_splits = [nn]
                a0 = 0
                for b0 in ((p0_splits if p0_split else [nn]) if nn else []):
                    nc.vector._custom_dve(HYP, out=ssum[:, a0:b0, :],
                                          in0=tin[:, a0:b0, :],
                                          in1=tin[:, nn + a0:nn + b0, :])
                    nc.scalar.activation(ssum[:, a0:b0, :],
                                         ssum[:, a0:b0, :], AF.Ln)
                    nc.scalar.activation(rn[:, a0:b0, :], ssum[:, a0:b0, :],
                                         AF.Exp, scale=-0.5)
                    a0 = b0
                return tin, tin8, rn

            def emit_steps(t, F, tin, tin8, rn):
                tout = iop.tile([P, c_out, F], f16, tag="tout", name="tout")
                nt = wp.tile([P, 2 * mm, F], f16, tag="nt",
                             name="nt") if mm else None
                rnx = wp.tile([P, s, F], f16, tag="rnx",
                              name="rnx") if s else None
                rny = wp.tile([P, s, F], f16, tag="rny",
                              name="rny") if s else None

                # per-slider-pair rn scaled by slider length (x/y flavors)
                for j, k in enumerate(S):
                    nc.vector.tensor_scalar_mul(
                        rnx[:, j, :], rn[:, r0 + j, :], c["slnx"][k])
                    nc.vector.tensor_scalar_mul(
                        rny[:, j, :], rn[:, r0 + j, :], c["slny"][k])

                # materialize nt: wall pairs (unit scale), slider pairs
                # (pre-scaled); interleaved [c0 s0 c1 s1 ...]
                if r0:
                    nc.vector.tensor_mul(nt[:, 0:2 * r0:2, :],
                                         tin[:, 0:r0, :], rn[:, 0:r0, :])
                    nc.vector.tensor_mul(nt[:, 1:2 * r0:2, :],
                                         tin[:, nn:nn + r0, :], rn[:, 0:r0, :])
                if s:
                    nc.vector.tensor_mul(nt[:, 2 * r0:2 * mm:2, :],
                                         tin[:, r0:r0 + s, :], rnx[:])
                    nc.vector.tensor_mul(nt[:, 2 * r0 + 1:2 * mm:2, :],
                                         tin[:, nn + r0:nn + r0 + s, :],
                                         rny[:])

                # rerand c0c1 = 0.5*v + 0.5 (run-grouped, even/odd slots);
                # int8 source runs fold the dequant into the ACT scale.
                rer_sets = [(lay["rer_runs16"], tin, rer_off, 0.5,
                             rer16_eng or rer_eng)]
                if use_i8:
                    rer_sets.append(
                        (lay["rer_runs8"], tin8, 0, 0.5 * q_scale, rer_eng))
                for runs, srct, off0, scl, rer_eng_ in rer_sets:
                    for (i0, n, c0, st) in runs:
                        for par in (0, 1):
                            src = srct[:, off0 + 2 * i0 + par:
                                       off0 + 2 * (i0 + n):2, :]
                            dst = tout[:, c0 + par:
                                       c0 + par + st * (n - 1) + 1:st, :] \
                                if n > 1 else tout[:, c0 + par, :]
                            if n == 1:
                                src = srct[:, off0 + 2 * i0 + par, :]
                            if rer_eng_ == "act":
                                nc.scalar.activation(dst, src, AF.Copy,
                                                     bias=0.5, scale=scl)
                            elif rer_eng_ == "pool":
                                nc.gpsimd.tensor_scalar(
                                    dst, src, scl, 0.5,
                                    mybir.AluOpType.mult,
                                    mybir.AluOpType.add)
                            else:
                                nc.vector.tensor_scalar(
                                    dst, src, scl, 0.5,
                                    mybir.AluOpType.mult,
                                    mybir.AluOpType.add)

                # circle&rerand c2c3 = normalized high pair (run-grouped muls)
                cr1_base = r0 + s
                for (i0, n, c2, st) in lay["cr1_runs"]:
                    for par in (0, 1):
                        if cr1_eng == "mix":
                            cr1_mul = (nc.gpsimd.tensor_mul if par == 0
                                       else nc.vector.tensor_mul)
                        else:
                            cr1_mul = (nc.gpsimd.tensor_mul
                                       if cr1_eng == "pool"
                                       else nc.vector.tensor_mul)
                        blk = (0 if par == 0 else nn)
                        src = tin[:, blk + cr1_base + i0:
                                  blk + cr1_base + i0 + n, :]
                        rsr = rn[:, cr1_base + i0:cr1_base + i0 + n, :]
                        dst = tout[:, c2 + par:
                                   c2 + par + st * (n - 1) + 1:st, :] \
                            if n > 1 else tout[:, c2 + par, :]
                        if n == 1:
                            src = tin[:, blk + cr1_base + i0, :]
                            rsr = rn[:, cr1_base + i0, :]
                        cr1_mul(dst, src, rsr)

                # circle&wall c2c3 = normalized low pair (copy from nt)
                for k in CR0:
                    i = wall_idx[k]
                    dst = tout[:, col_of[(k, 2)]:col_of[(k, 2)] + 2, :]
                    srcp = nt[:, 2 * i:2 * i + 2, :]
                    if cr0_eng == "pool":
                        nc.gpsimd.tensor_copy(dst, srcp)
                    elif cr0_eng == "act":
                        nc.scalar.activation(dst, srcp, AF.Copy)
                    else:
                        nc.vector.tensor_copy(dst, srcp)

                def emit_slider(k):
                    j = slider_idx[k]
                    A = nt[:, 2 * (r0 + j), :]       # chat*slnx
                    Bv = nt[:, 2 * (r0 + j) + 1, :]  # shat*slny
                    ax, ay = c["slnx"][k], c["slny"][k]
                    nc.vector._custom_dve(
                        LIN, out=tout[:, col_of[(k, 2)], :], in0=A, in1=Bv,
                        s0=c["scos"][k] / ax, s1=-c["ssin"][k] / ay)
                    nc.vector._custom_dve(
                        LIN, out=tout[:, col_of[(k, 3)], :], in0=A, in1=Bv,
                        s0=c["ssin"][k] / ax, s1=c["scos"][k] / ay)
                    add_op = (nc.gpsimd.tensor_add if adds_eng == "pool"
                              else nc.vector.tensor_add)
                    add_op(tout[:, col_of[(k, 4)], :],
                           tout[:, col_of[(k, 0)], :], A)
                    add_op(tout[:, col_of[(k, 5)], :],
                           tout[:, col_of[(k, 1)], :], Bv)

                # rerand-slider outputs don't depend on walls: emit early
                for k in S:
                    if c["rr"][k]:
                        emit_slider(k)

                # wall steps: folded clamp, ascending k (carry chain)
                for k in W:
                    i = wall_idx[k]
                    if k == 0:
                        pxs, pys = px0t[:, 0:F], py0t[:, 0:F]
                    else:
                        pxs = tout[:, col_of[(k - 1, 0)], :]
                        pys = tout[:, col_of[(k - 1, 1)], :]
                    c0 = tout[:, col_of[(k, 0)], :]
                    c1 = tout[:, col_of[(k, 1)], :]
                    nc.vector._custom_dve(Q1, out=c0, in0=pxs,
                                          in1=nt[:, 2 * i, :],
                                          s0=c["wl"][k], s1=0.5,
                                          imm2=-2.0 * c["lkx"][k])
                    nc.vector._custom_dve(Q2, out=c0, in0=c0,
                                          in1=nt[:, 2 * i, :],
                                          s0=c["wr"][k], s1=0.5,
                                          imm2=2.0 * c["lkx"][k])
                    nc.vector._custom_dve(Q1, out=c1, in0=pys,
                                          in1=nt[:, 2 * i + 1, :],
                                          s0=c["wt"][k], s1=0.5,
                                          imm2=-2.0 * c["lky"][k])
                    nc.vector._custom_dve(Q2, out=c1, in0=c1,
                                          in1=nt[:, 2 * i + 1, :],
                                          s0=c["wb"][k], s1=0.5,
                                          imm2=2.0 * c["lky"][k])

                # wall-slider outputs (need this step's clamped c0c1)
                for k in S:
                    if not c["rr"][k]:
                        emit_slider(k)

                if out_split <= 1:
                    nc.sync.dma_start(out[t * P:(t + 1) * P, :, 0:F],
                                      tout[:])
                else:
                    a = 0
                    ochunks = (out_chunks or lay["out_chunks"])
                    if out_chunks_last is not None and t == n_tiles - 1:
                        ochunks = out_chunks_last
                    for b in ochunks:
                        nc.sync.dma_start(
                            out[t * P:(t + 1) * P, a:b, 0:F],
                            tout[:, a:b, :])
                        a = b

            if p0_first:
                handles = [emit_phase0(t, F) for t, F in enumerate(Fs)]
                for t, F in enumerate(Fs):
                    emit_steps(t, F, *handles[t])
            else:
                for t, F in enumerate(Fs):
                    emit_steps(t, F, *emit_phase0(t, F))
    nc.compile()
    return nc


def kernel(**inputs):
    var = np.asarray(inputs["var_tensor"], dtype=np.float32)
    B = var.shape[0]
    assert B % (N_CORES * P) == 0
    b_core = B // N_CORES
    c = _host_consts(
        np.asarray(inputs["slider_lengths"]), np.asarray(inputs["slider_cos_each"]),
        np.asarray(inputs["slider_sin_each"]), np.asarray(inputs["note_distances"]),
        np.asarray(inputs["tick_diff"]), np.asarray(inputs["start_pos"]),
        np.asarray(inputs["is_slider"]))
    lay = _layout(c)
    npp = b_core // P
    c_in, c_in8, c_out = lay["c_in"], lay["c_in8"], lay["c_out"]

    # int8 quantization scale for the carry-free rerand inputs
    q_scale = None
    if c_in8:
        vmax = float(np.abs(var[:, lay["in8_perm"]]).max())
        q_scale = max(vmax, 1e-3) / 126.0

    def tiles_for(n_tiles):
        base, rem = divmod(npp, n_tiles)
        return [base + (1 if t < rem else 0) for t in range(n_tiles)]

    oc = [8, 14, 24, 44] if c_out == 44 else None
    # tuned config first; progressively conservative fallbacks for layouts
    # whose working set overflows SBUF (e.g. many slider/wall steps)
    attempts = [
        (4, dict(in_bufs=4, cr1_eng="pool", work_bufs=2, io_bufs=3,
                 out_split=3, out_chunks=oc, cr0_eng="pool",
                 rer16_eng="vector", p0_first=True)),
        (4, dict(in_bufs=2, cr1_eng="pool", work_bufs=2, io_bufs=2,
                 out_split=1, cr0_eng="pool")),
        (8, dict(in_bufs=2, cr1_eng="pool", work_bufs=2, io_bufs=2,
                 out_split=1, cr0_eng="pool")),
        (16, dict(in_bufs=2, work_bufs=2, io_bufs=2, out_split=1)),
    ]
    key = (B, tuple(sorted((k, v) for k, v in c.items())),
           None if q_scale is None else round(q_scale, 9))
    if key not in _NC_CACHE:
        err = None
        for n_tiles_a, kw in attempts:
            try:
                fs_a = tiles_for(n_tiles_a)
                ncb = _build(c, lay, b_core, fs=fs_a, q_scale=q_scale, **kw)
                _NC_CACHE[key] = ncb
                _FS_CACHE[key] = fs_a
                err = None
                break
            except Exception as e:
                err = e
        if err is not None:
            raise err
    nc = _NC_CACHE[key]
    Fs = _FS_CACHE[key]
    n_tiles = len(Fs)
    Fmax = max(Fs)

    # ---- stage inputs: per-core [T*P, C, Fmax] column-major tiles ----
    var16 = var.astype(np.float16)
    if c_in8:
        var8 = np.clip(np.round(var[:, lay["in8_perm"]] / q_scale),
                       -127, 127).astype(np.int8)
    in_maps = []
    for ci in range(N_CORES):
        arr = var16[ci * b_core:(ci + 1) * b_core]          # [b_core, 40]
        arr = arr.reshape(P, npp, 40)
        staged = np.zeros((n_tiles, P, c_in, Fmax), dtype=np.float16)
        if c_in8:
            arr8 = var8[ci * b_core:(ci + 1) * b_core].reshape(P, npp, c_in8)
            staged8 = np.zeros((n_tiles, P, c_in8, Fmax), dtype=np.int8)
        off = 0
        for t, F in enumerate(Fs):
            blk = arr[:, off:off + F, :][:, :, lay["in_perm"]]  # [P,F,C]
            staged[t, :, :, :F] = blk.transpose(0, 2, 1)
            if c_in8:
                staged8[t, :, :, :F] = arr8[:, off:off + F, :].transpose(
                    0, 2, 1)
            off += F
        im = {"var": staged.reshape(n_tiles * P, c_in, Fmax)}
        if c_in8:
            im["var8"] = staged8.reshape(n_tiles * P, c_in8, Fmax)
        in_maps.append(im)

    from concourse.bass_utils import run_bass_kernel_spmd
    res = run_bass_kernel_spmd(nc, in_maps, core_ids=list(range(N_CORES)))

    # ---- unstage: device cols -> [B, NGS, 6]; fan out circle c4c5 ----
    out = np.empty((B, NGS, 6), dtype=np.float32)
    for ci in range(N_CORES):
        staged = np.asarray(res.results[ci]["out"]).reshape(
            n_tiles, P, c_out, Fmax)
        core_rows = np.empty((P, npp, c_out), dtype=np.float16)
        off = 0
        for t, F in enumerate(Fs):
            core_rows[:, off:off + F, :] = \
                staged[t, :, :, :F].transpose(0, 2, 1)
            off += F
        flat = core_rows.reshape(b_core, c_out).astype(np.float32)
        dst = out[ci * b_core:(ci + 1) * b_core]
        for (k, m) in lay["dev_cols"]:
            dst[:, k, m] = flat[:, lay["col_of"][(k, m)]]
        for k in lay["CR1"] + lay["CR0"]:
            dst[:, k, 4] = dst[:, k, 0]
            dst[:, k, 5] = dst[:, k, 1]
    return out
